# revision 12
# baseline (speedup 1.0000x reference)
"""Causal self-attention (B=2, T=2048, D=1024, H=16) on 8 trn2 cores.

Sharding: tensor-parallel over heads x data-parallel over batch.
Core c handles batch b = c // 4, head group g = c % 4 (heads 4g..4g+3).
Host pre-slices/pre-transposes weight+activation shards (cast to bf16);
each core returns a partial y (its heads' contribution); host sums
groups of 4.

Kernel structure (all matmuls bf16, psum f32):
  per t-tile tt (512 wide):
    A(tt): qkv projection for that t-slice (ci-outer so input DMA is
           consumed as it arrives)
    outproj(tt-1): output projection of the previous tile (deferred one
           tile so its dependency on the softmax normalization is long
           resolved -> no tensor stall)
    B(tt, pr) for each head pair: attention with a depth-1 software
           pipeline (QK(ss) issued before PV(ss-1)) so TensorE never
           waits on the Scalar-engine exp; both heads of the pair share
           one [128,2,512] psum quad -> one exp instruction per s-chunk.
    norm(tt, pr): 1/L via DVE reciprocal + gpsimd partition_broadcast +
           DVE multiply (no TensorE involvement).
  Causal masking: s-chunks beyond the diagonal are skipped entirely;
  diagonal chunks stream only the t >= s columns (partial-width matmuls)
  plus an affine_select for the 128-wide triangle.
"""

import os
import sys

for _p in ("/opt/trn_rl_repo", "/root/.axon_site/_ro/trn_rl_repo"):
    if os.path.isdir(_p) and _p not in sys.path:
        sys.path.insert(0, _p)

import ml_dtypes
import numpy as np

import concourse.bass as bass
import concourse.mybir as mybir
import concourse.tile as tile
from concourse import bacc
from concourse.bass_utils import run_bass_kernel_spmd

F32 = mybir.dt.float32
BF16 = mybir.dt.bfloat16
U16 = mybir.dt.uint16

B, T, C = 2, 2048, 1024
NHEAD_TOT = 16
DH = 64
NCORES = 8
NH = 4          # heads per core
NPAIR = 2       # head pairs per core
CK = C // 128   # contraction chunks (8)
TT = 512        # t-tile width
NTT = T // TT   # 4
FQK = 2 * NH * DH  # 512 cols of qkv^T for q+k
FV = NH * DH       # 256 cols for v
ONE_BF16 = 0x3F80


def build_nc():
    nc = bacc.Bacc("TRN2", target_bir_lowering=False, debug=False)

    xT = nc.dram_tensor("xT", [C, T], BF16, kind="ExternalInput")
    wqkvT = nc.dram_tensor("wqkvT", [C, FQK + FV], BF16, kind="ExternalInput")
    woutT = nc.dram_tensor("woutT", [NH * DH, C], BF16, kind="ExternalInput")
    y = nc.dram_tensor("y", [T, C], BF16, kind="ExternalOutput")

    EXP = mybir.ActivationFunctionType.Exp

    with tile.TileContext(nc) as tc:
        with (
            tc.tile_pool(name="const", bufs=1) as const,
            tc.tile_pool(name="ptp", bufs=4) as ptp,
            tc.tile_pool(name="bcp", bufs=4) as bcp,
            tc.tile_pool(name="rcp", bufs=4) as rcp,
            tc.tile_pool(name="yp", bufs=2) as yp,
            tc.tile_pool(name="psS", bufs=2, space="PSUM") as psS,
            tc.tile_pool(name="psV", bufs=4, space="PSUM") as psV,
        ):
            # ---- persistent SBUF ----
            xT_sb = const.tile([128, CK, T], BF16)            # x^T (c-major)
            wqkvT_sb = const.tile([128, CK, FQK + FV], BF16)  # cols [q(4x64)|k(4x64)|v(4x64)]
            woutT_sb = const.tile([128, NPAIR, C], BF16)      # W_out^T rows per head pair
            qk_t = [const.tile([128, 4, TT], BF16, name=f"qk_t{i}") for i in range(NTT)]  # [qp0|qp1|kp0|kp1]
            v_t = [const.tile([128, 4, NH, DH + 64], BF16, name=f"v_t{i}") for i in range(NTT)]  # V + 64 ones cols
            o_t = [const.tile([128, NPAIR, TT], BF16, name=f"o_t{i}") for i in range(NTT)]  # normalized O^T

            for tt in range(NTT):
                nc.vector.memset(v_t[tt][:, :, :, DH:DH + 64].bitcast(U16), ONE_BF16)

            # ---- DMAs: first tile's deps first ----
            for ci in range(CK):
                nc.sync.dma_start(wqkvT_sb[:, ci, :], wqkvT[ci * 128:(ci + 1) * 128, :])
                nc.sync.dma_start(xT_sb[:, ci, 0:TT], xT[ci * 128:(ci + 1) * 128, 0:TT])
            for tt in range(1, NTT):
                for ci in range(CK):
                    nc.sync.dma_start(xT_sb[:, ci, tt * TT:(tt + 1) * TT],
                                      xT[ci * 128:(ci + 1) * 128, tt * TT:(tt + 1) * TT])
            for pr in range(NPAIR):
                nc.sync.dma_start(woutT_sb[:, pr, :], woutT[pr * 128:(pr + 1) * 128, :])

            def phase_a(tt):
                """QKV projection for t-tile tt."""
                tsl = slice(tt * TT, (tt + 1) * TT)
                for fq in range(2):  # 0: q pairs, 1: k pairs
                    ps = psS.tile([128, 2, TT], F32, tag="ps", name=f"qk{tt}_{fq}")
                    for f2 in range(2):
                        f = 2 * fq + f2
                        for ci in range(CK):
                            nc.tensor.matmul(
                                ps[:, f2, :],
                                wqkvT_sb[:, ci, f * 128:(f + 1) * 128],
                                xT_sb[:, ci, tsl],
                                start=(ci == 0), stop=(ci == CK - 1),
                            )
                    nc.vector.tensor_copy(qk_t[tt][:, 2 * fq:2 * fq + 2, :], ps)
                for sp in range(2):
                    pvv = [psS.tile([128, FV], F32, tag="ps", name=f"v{tt}_{sp}_{k}")
                           for k in range(2)]
                    for k in range(2):
                        si = tt * 4 + sp * 2 + k
                        for ci in range(CK):
                            nc.tensor.matmul(
                                pvv[k],
                                xT_sb[:, ci, si * 128:(si + 1) * 128],
                                wqkvT_sb[:, ci, FQK:FQK + FV],
                                start=(ci == 0), stop=(ci == CK - 1),
                            )
                    for k in range(2):
                        nc.vector.tensor_copy(
                            v_t[tt][:, sp * 2 + k, :, 0:DH],
                            pvv[k].rearrange("p (h d) -> p h d", h=NH),
                        )

            def phase_b(tt, pr, pending):
                """Attention for (t-tile, head pair), depth-1 pipelined.
                `pending` (deferred norm emitter) is flushed after the first
                QK+exp so its broadcast matmul never stalls the PE queue."""
                n_ss = 4 * (tt + 1)
                pv = [psV.tile([128, TT], F32, tag="pv", name=f"pv{tt}_{pr}_{hi}")
                      for hi in range(2)]

                def emit_pv(pt, t0, ss):
                    for hi in range(2):
                        nc.tensor.matmul(
                            pv[hi][:, t0:TT],
                            v_t[ss // 4][:, ss % 4, pr * 2 + hi, :],
                            pt[:, hi, t0:TT],
                            start=(ss == 0), stop=(ss == n_ss - 1),
                            skip_group_check=True,
                        )

                prev = None
                for ss in range(n_ss):
                    t0 = max(0, 128 * ss - TT * tt)
                    ps = psS.tile([128, 2, TT], F32, tag="ps", name=f"s{tt}_{pr}_{ss}")
                    for hi in range(2):
                        nc.tensor.matmul(
                            ps[:, hi, t0:TT],
                            qk_t[ss // 4][hi * 64:(hi + 1) * 64, 2 + pr,
                                          (ss % 4) * 128:(ss % 4 + 1) * 128],
                            qk_t[tt][hi * 64:(hi + 1) * 64, pr, t0:TT],
                        )
                    pt = ptp.tile([128, 2, TT], BF16, tag="pt", name=f"pt{tt}_{pr}_{ss}")
                    nc.scalar.activation(pt[:, :, t0:TT], ps[:, :, t0:TT], EXP, scale=0.125)
                    if ss >= 4 * tt:  # diagonal chunk: zero the s > t triangle
                        for hi in range(2):
                            nc.gpsimd.affine_select(
                                out=pt[:, hi, t0:t0 + 128],
                                in_=pt[:, hi, t0:t0 + 128],
                                compare_op=mybir.AluOpType.is_ge,
                                fill=0.0,
                                base=0,
                                channel_multiplier=-1,
                                pattern=[[1, 128]],
                            )
                    if pending is not None and ss == 1:
                        pending()
                        pending = None
                    if prev is not None:
                        emit_pv(*prev)
                    prev = (pt, t0, ss)
                emit_pv(*prev)
                if pending is not None:
                    pending()
                return pv

            def norm(tt, pr, pv):
                """o = pv[0:64] / L; pv[64:128] all hold L (64 ones cols in
                v), so one 64-wide reciprocal IS the partition broadcast."""
                for hi in range(2):
                    rc = rcp.tile([64, TT], BF16, tag="rcr", name=f"rc{tt}_{pr}_{hi}")
                    with nc.allow_low_precision("bf16 softmax denominators"):
                        nc.vector.reciprocal(rc, pv[hi][64:128, :])
                    nc.vector.tensor_mul(
                        o_t[tt][hi * 64:(hi + 1) * 64, pr, :],
                        pv[hi][0:DH, :],
                        rc,
                    )

            def outproj(tt):
                """y[tt-slice] = sum_pr o_t[tt]^T @ woutT."""
                for tq in range(4):
                    ps = psS.tile([128, 2, TT], F32, tag="ps", name=f"y{tt}_{tq}")
                    for ot in range(2):
                        for pr in range(NPAIR):
                            nc.tensor.matmul(
                                ps[:, ot, :],
                                o_t[tt][:, pr, tq * 128:(tq + 1) * 128],
                                woutT_sb[:, pr, ot * TT:(ot + 1) * TT],
                                start=(pr == 0), stop=(pr == NPAIR - 1),
                            )
                    yt = yp.tile([128, 2, TT], BF16, tag="yt", name=f"yt{tt}_{tq}")
                    nc.vector.tensor_copy(yt, ps)
                    row = tt * 4 + tq
                    nc.sync.dma_start(
                        y[row * 128:(row + 1) * 128, :],
                        yt.rearrange("p a b -> p (a b)"),
                    )

            pending = None
            for tt in range(NTT):
                phase_a(tt)
                pv0 = phase_b(tt, 0, pending)  # flushes norm(tt-1, 1)
                if tt > 0:
                    outproj(tt - 1)
                pending = (lambda a=tt, b=pv0: norm(a, 0, b))
                pv1 = phase_b(tt, 1, pending)  # flushes norm(tt, 0)
                pending = (lambda a=tt, b=pv1: norm(a, 1, b))
            pending()
            outproj(NTT - 1)

    nc.compile()
    return nc


_NC_CACHE = None


def _get_nc():
    global _NC_CACHE
    if _NC_CACHE is None:
        _NC_CACHE = build_nc()
    return _NC_CACHE


def make_in_maps(x, W_qkv, W_out):
    bf = ml_dtypes.bfloat16
    x = np.asarray(x, dtype=np.float32)
    W_qkv = np.asarray(W_qkv, dtype=np.float32)
    W_out = np.asarray(W_out, dtype=np.float32)
    xT = [np.ascontiguousarray(x[b].T.astype(bf)) for b in range(B)]
    in_maps = []
    for c in range(NCORES):
        b, g = c // 4, c % 4
        rq = W_qkv[g * 256:(g + 1) * 256]            # q rows, heads 4g..4g+3
        rk = W_qkv[C + g * 256:C + (g + 1) * 256]    # k rows
        rv = W_qkv[2 * C + g * 256:2 * C + (g + 1) * 256]  # v rows
        wqkvT = np.ascontiguousarray(
            np.concatenate([rq, rk, rv], axis=0).T.astype(bf))
        woutT = np.ascontiguousarray(W_out[:, g * 256:(g + 1) * 256].T.astype(bf))
        in_maps.append({"xT": xT[b], "wqkvT": wqkvT, "woutT": woutT})
    return in_maps


def kernel(x, W_qkv, W_out):
    nc = _get_nc()
    in_maps = make_in_maps(x, W_qkv, W_out)
    res = run_bass_kernel_spmd(nc, in_maps, core_ids=list(range(NCORES)))
    kernel.last_results = res
    y = np.zeros((B, T, C), dtype=np.float32)
    for c in range(NCORES):
        y[c // 4] += res.results[c]["y"].astype(np.float32)
    return y


# revision 14
# speedup vs baseline: 1.1828x; 1.1828x over previous
"""Causal self-attention (B=2, T=2048, D=1024, H=16) on 8 trn2 cores.

Sharding: tensor-parallel over heads x data-parallel over batch.
Core c handles batch b = c // 4, head group g = c % 4 (heads 4g..4g+3).
Host pre-slices/pre-transposes weight+activation shards (cast to bf16);
each core returns a partial y (its heads' contribution); host sums
groups of 4.

Kernel structure (all matmuls bf16, psum f32):
  per t-tile tt (512 wide):
    A(tt): qkv projection for that t-slice (ci-outer so input DMA is
           consumed as it arrives)
    outproj(tt-1): output projection of the previous tile (deferred one
           tile so its dependency on the softmax normalization is long
           resolved -> no tensor stall)
    B(tt, pr) for each head pair: attention with a depth-1 software
           pipeline (QK(ss) issued before PV(ss-1)) so TensorE never
           waits on the Scalar-engine exp; both heads of the pair share
           one [128,2,512] psum quad -> one exp instruction per s-chunk.
    norm(tt, pr): 1/L via DVE reciprocal + gpsimd partition_broadcast +
           DVE multiply (no TensorE involvement).
  Causal masking: s-chunks beyond the diagonal are skipped entirely;
  diagonal chunks stream only the t >= s columns (partial-width matmuls)
  plus an affine_select for the 128-wide triangle.
"""

import os
import sys

for _p in ("/opt/trn_rl_repo", "/root/.axon_site/_ro/trn_rl_repo"):
    if os.path.isdir(_p) and _p not in sys.path:
        sys.path.insert(0, _p)

import ml_dtypes
import numpy as np

import concourse.bass as bass
import concourse.mybir as mybir
import concourse.tile as tile
from concourse import bacc
from concourse.bass_utils import run_bass_kernel_spmd

F32 = mybir.dt.float32
BF16 = mybir.dt.bfloat16
U16 = mybir.dt.uint16

B, T, C = 2, 2048, 1024
NHEAD_TOT = 16
DH = 64
NCORES = 8
NH = 4          # heads per core
NPAIR = 2       # head pairs per core
CK = C // 128   # contraction chunks (8)
TT = 512        # t-tile width
NTT = T // TT   # 4
FQK = 2 * NH * DH  # 512 cols of qkv^T for q+k
FV = NH * DH       # 256 cols for v
ONE_BF16 = 0x3F80


def build_nc():
    nc = bacc.Bacc("TRN2", target_bir_lowering=False, debug=False)

    xT = nc.dram_tensor("xT", [C, T], BF16, kind="ExternalInput")
    wqkvT = nc.dram_tensor("wqkvT", [C, FQK + FV], BF16, kind="ExternalInput")
    woutT = nc.dram_tensor("woutT", [NH * DH, C], BF16, kind="ExternalInput")
    y = nc.dram_tensor("y", [T, C], BF16, kind="ExternalOutput")

    EXP = mybir.ActivationFunctionType.Exp

    with tile.TileContext(nc) as tc:
        with (
            tc.tile_pool(name="const", bufs=1) as const,
            tc.tile_pool(name="ptp", bufs=4) as ptp,
            tc.tile_pool(name="bcp", bufs=4) as bcp,
            tc.tile_pool(name="rcp", bufs=4) as rcp,
            tc.tile_pool(name="yp", bufs=2) as yp,
            tc.tile_pool(name="psS", bufs=2, space="PSUM") as psS,
            tc.tile_pool(name="psV", bufs=4, space="PSUM") as psV,
        ):
            # ---- persistent SBUF ----
            xT_sb = const.tile([128, CK, T], BF16)            # x^T (c-major)
            wqkvT_sb = const.tile([128, CK, FQK + FV], BF16)  # cols [q(4x64)|k(4x64)|v(4x64)]
            woutT_sb = const.tile([128, NPAIR, C], BF16)      # W_out^T rows per head pair
            qk_t = [const.tile([128, 4, TT], BF16, name=f"qk_t{i}") for i in range(NTT)]  # [qp0|qp1|kp0|kp1]
            v_t = [const.tile([128, 4, NH, DH + 64], BF16, name=f"v_t{i}") for i in range(NTT)]  # V + 64 ones cols
            o_t = [const.tile([128, NPAIR, TT], BF16, name=f"o_t{i}") for i in range(NTT)]  # normalized O^T

            for tt in range(NTT):
                nc.vector.memset(v_t[tt][:, :, :, DH:DH + 64].bitcast(U16), ONE_BF16)

            # ---- DMAs: first tile's deps first ----
            for ci in range(CK):
                nc.sync.dma_start(wqkvT_sb[:, ci, :], wqkvT[ci * 128:(ci + 1) * 128, :])
                nc.sync.dma_start(xT_sb[:, ci, 0:TT], xT[ci * 128:(ci + 1) * 128, 0:TT])
            for tt in range(1, NTT):
                for ci in range(CK):
                    nc.sync.dma_start(xT_sb[:, ci, tt * TT:(tt + 1) * TT],
                                      xT[ci * 128:(ci + 1) * 128, tt * TT:(tt + 1) * TT])
            for pr in range(NPAIR):
                nc.sync.dma_start(woutT_sb[:, pr, :], woutT[pr * 128:(pr + 1) * 128, :])

            def phase_a(tt):
                """QKV projection for t-tile tt."""
                tsl = slice(tt * TT, (tt + 1) * TT)
                for fq in range(2):  # 0: q pairs, 1: k pairs
                    ps = psS.tile([128, 2, TT], F32, tag="ps", name=f"qk{tt}_{fq}")
                    for f2 in range(2):
                        f = 2 * fq + f2
                        for ci in range(CK):
                            nc.tensor.matmul(
                                ps[:, f2, :],
                                wqkvT_sb[:, ci, f * 128:(f + 1) * 128],
                                xT_sb[:, ci, tsl],
                                start=(ci == 0), stop=(ci == CK - 1),
                            )
                    nc.vector.tensor_copy(qk_t[tt][:, 2 * fq:2 * fq + 2, :], ps)
                for sp in range(2):
                    pvv = [psS.tile([128, FV], F32, tag="ps", name=f"v{tt}_{sp}_{k}")
                           for k in range(2)]
                    for k in range(2):
                        si = tt * 4 + sp * 2 + k
                        for ci in range(CK):
                            nc.tensor.matmul(
                                pvv[k],
                                xT_sb[:, ci, si * 128:(si + 1) * 128],
                                wqkvT_sb[:, ci, FQK:FQK + FV],
                                start=(ci == 0), stop=(ci == CK - 1),
                            )
                    for k in range(2):
                        nc.vector.tensor_copy(
                            v_t[tt][:, sp * 2 + k, :, 0:DH],
                            pvv[k].rearrange("p (h d) -> p h d", h=NH),
                        )

            def phase_b(tt, pr, pending):
                """Attention for (t-tile, head pair), depth-1 pipelined.
                `pending` (deferred norm emitter) is flushed after the first
                QK+exp so its broadcast matmul never stalls the PE queue."""
                n_ss = 4 * (tt + 1)
                pv = [psV.tile([128, TT], F32, tag="pv", name=f"pv{tt}_{pr}_{hi}")
                      for hi in range(2)]

                def emit_pv(pt, t0, ss):
                    for hi in range(2):
                        nc.tensor.matmul(
                            pv[hi][:, t0:TT],
                            v_t[ss // 4][:, ss % 4, pr * 2 + hi, :],
                            pt[:, hi, t0:TT],
                            start=(ss == 0), stop=(ss == n_ss - 1),
                            skip_group_check=True,
                        )

                prev = None
                for ss in range(n_ss):
                    t0 = max(0, 128 * ss - TT * tt)
                    ps = psS.tile([128, 2, TT], F32, tag="ps", name=f"s{tt}_{pr}_{ss}")
                    for hi in range(2):
                        nc.tensor.matmul(
                            ps[:, hi, t0:TT],
                            qk_t[ss // 4][hi * 64:(hi + 1) * 64, 2 + pr,
                                          (ss % 4) * 128:(ss % 4 + 1) * 128],
                            qk_t[tt][hi * 64:(hi + 1) * 64, pr, t0:TT],
                        )
                    pt = ptp.tile([128, 2, TT], BF16, tag="pt", name=f"pt{tt}_{pr}_{ss}")
                    nc.scalar.activation(pt[:, :, t0:TT], ps[:, :, t0:TT], EXP, scale=0.125)
                    if ss >= 4 * tt:  # diagonal chunk: zero the s > t triangle
                        for hi in range(2):
                            nc.gpsimd.affine_select(
                                out=pt[:, hi, t0:t0 + 128],
                                in_=pt[:, hi, t0:t0 + 128],
                                compare_op=mybir.AluOpType.is_ge,
                                fill=0.0,
                                base=0,
                                channel_multiplier=-1,
                                pattern=[[1, 128]],
                            )
                    if pending is not None and ss == 1:
                        pending()
                        pending = None
                    if prev is not None:
                        emit_pv(*prev)
                    prev = (pt, t0, ss)
                emit_pv(*prev)
                if pending is not None:
                    pending()
                return pv

            def norm(tt, pr, pv):
                """o = pv[0:64] / L; pv[64:128] all hold L (64 ones cols in
                v), so one 64-wide reciprocal IS the partition broadcast."""
                for hi in range(2):
                    lc = bcp.tile([64, TT], F32, tag="lc", name=f"lc{tt}_{pr}_{hi}")
                    nc.vector.tensor_copy(lc, pv[hi][64:128, :])
                    rc = rcp.tile([64, TT], F32, tag="rcr", name=f"rc{tt}_{pr}_{hi}")
                    nc.vector.reciprocal_approx_fast(out=rc, in_=lc)
                    nc.vector.tensor_mul(
                        o_t[tt][hi * 64:(hi + 1) * 64, pr, :],
                        pv[hi][0:DH, :],
                        rc,
                    )

            def outproj(tt):
                """y[tt-slice] = sum_pr o_t[tt]^T @ woutT."""
                for tq in range(4):
                    ps = psS.tile([128, 2, TT], F32, tag="ps", name=f"y{tt}_{tq}")
                    for ot in range(2):
                        for pr in range(NPAIR):
                            nc.tensor.matmul(
                                ps[:, ot, :],
                                o_t[tt][:, pr, tq * 128:(tq + 1) * 128],
                                woutT_sb[:, pr, ot * TT:(ot + 1) * TT],
                                start=(pr == 0), stop=(pr == NPAIR - 1),
                            )
                    yt = yp.tile([128, 2, TT], BF16, tag="yt", name=f"yt{tt}_{tq}")
                    nc.vector.tensor_copy(yt, ps)
                    row = tt * 4 + tq
                    nc.sync.dma_start(
                        y[row * 128:(row + 1) * 128, :],
                        yt.rearrange("p a b -> p (a b)"),
                    )

            pending = None
            for tt in range(NTT):
                phase_a(tt)
                pv0 = phase_b(tt, 0, pending)  # flushes norm(tt-1, 1)
                if tt > 0:
                    outproj(tt - 1)
                pending = (lambda a=tt, b=pv0: norm(a, 0, b))
                pv1 = phase_b(tt, 1, pending)  # flushes norm(tt, 0)
                pending = (lambda a=tt, b=pv1: norm(a, 1, b))
            pending()
            outproj(NTT - 1)

    nc.compile()
    return nc


_NC_CACHE = None


def _get_nc():
    global _NC_CACHE
    if _NC_CACHE is None:
        _NC_CACHE = build_nc()
    return _NC_CACHE


def make_in_maps(x, W_qkv, W_out):
    bf = ml_dtypes.bfloat16
    x = np.asarray(x, dtype=np.float32)
    W_qkv = np.asarray(W_qkv, dtype=np.float32)
    W_out = np.asarray(W_out, dtype=np.float32)
    xT = [np.ascontiguousarray(x[b].T.astype(bf)) for b in range(B)]
    in_maps = []
    for c in range(NCORES):
        b, g = c // 4, c % 4
        rq = W_qkv[g * 256:(g + 1) * 256]            # q rows, heads 4g..4g+3
        rk = W_qkv[C + g * 256:C + (g + 1) * 256]    # k rows
        rv = W_qkv[2 * C + g * 256:2 * C + (g + 1) * 256]  # v rows
        wqkvT = np.ascontiguousarray(
            np.concatenate([rq, rk, rv], axis=0).T.astype(bf))
        woutT = np.ascontiguousarray(W_out[:, g * 256:(g + 1) * 256].T.astype(bf))
        in_maps.append({"xT": xT[b], "wqkvT": wqkvT, "woutT": woutT})
    return in_maps


def kernel(x, W_qkv, W_out):
    nc = _get_nc()
    in_maps = make_in_maps(x, W_qkv, W_out)
    res = run_bass_kernel_spmd(nc, in_maps, core_ids=list(range(NCORES)))
    kernel.last_results = res
    y = np.zeros((B, T, C), dtype=np.float32)
    for c in range(NCORES):
        y[c // 4] += res.results[c]["y"].astype(np.float32)
    return y


# revision 15
# speedup vs baseline: 1.2084x; 1.0216x over previous
"""Causal self-attention (B=2, T=2048, D=1024, H=16) on 8 trn2 cores.

Sharding: tensor-parallel over heads x data-parallel over batch.
Core c handles batch b = c // 4, head group g = c % 4 (heads 4g..4g+3).
Host pre-slices/pre-transposes weight+activation shards (cast to bf16);
each core returns a partial y (its heads' contribution); host sums
groups of 4.

Kernel structure (all matmuls bf16, psum f32):
  per t-tile tt (512 wide):
    A(tt): qkv projection for that t-slice (ci-outer so input DMA is
           consumed as it arrives)
    outproj(tt-1): output projection of the previous tile (deferred one
           tile so its dependency on the softmax normalization is long
           resolved -> no tensor stall)
    B(tt, pr) for each head pair: attention with a depth-1 software
           pipeline (QK(ss) issued before PV(ss-1)) so TensorE never
           waits on the Scalar-engine exp; both heads of the pair share
           one [128,2,512] psum quad -> one exp instruction per s-chunk.
    norm(tt, pr): 1/L via DVE reciprocal + gpsimd partition_broadcast +
           DVE multiply (no TensorE involvement).
  Causal masking: s-chunks beyond the diagonal are skipped entirely;
  diagonal chunks stream only the t >= s columns (partial-width matmuls)
  plus an affine_select for the 128-wide triangle.
"""

import os
import sys

for _p in ("/opt/trn_rl_repo", "/root/.axon_site/_ro/trn_rl_repo"):
    if os.path.isdir(_p) and _p not in sys.path:
        sys.path.insert(0, _p)

import ml_dtypes
import numpy as np

import concourse.bass as bass
import concourse.mybir as mybir
import concourse.tile as tile
from concourse import bacc
from concourse.bass_utils import run_bass_kernel_spmd

F32 = mybir.dt.float32
BF16 = mybir.dt.bfloat16
U16 = mybir.dt.uint16

B, T, C = 2, 2048, 1024
NHEAD_TOT = 16
DH = 64
NCORES = 8
NH = 4          # heads per core
NPAIR = 2       # head pairs per core
CK = C // 128   # contraction chunks (8)
TT = 512        # t-tile width
NTT = T // TT   # 4
FQK = 2 * NH * DH  # 512 cols of qkv^T for q+k
FV = NH * DH       # 256 cols for v
ONE_BF16 = 0x3F80


def build_nc():
    nc = bacc.Bacc("TRN2", target_bir_lowering=False, debug=False)

    xT = nc.dram_tensor("xT", [C, T], BF16, kind="ExternalInput")
    wqkvT = nc.dram_tensor("wqkvT", [C, FQK + FV], BF16, kind="ExternalInput")
    woutT = nc.dram_tensor("woutT", [NH * DH, C], BF16, kind="ExternalInput")
    y = nc.dram_tensor("y", [T, C], BF16, kind="ExternalOutput")

    EXP = mybir.ActivationFunctionType.Exp

    with tile.TileContext(nc) as tc:
        with (
            tc.tile_pool(name="const", bufs=1) as const,
            tc.tile_pool(name="ptp", bufs=4) as ptp,
            tc.tile_pool(name="bcp", bufs=4) as bcp,
            tc.tile_pool(name="rcp", bufs=4) as rcp,
            tc.tile_pool(name="yp", bufs=2) as yp,
            tc.tile_pool(name="psS", bufs=2, space="PSUM") as psS,
            tc.tile_pool(name="psV", bufs=4, space="PSUM") as psV,
        ):
            # ---- persistent SBUF ----
            xT_sb = const.tile([128, CK, T], BF16)            # x^T (c-major)
            wqkvT_sb = const.tile([128, CK, FQK + FV], BF16)  # cols [q(4x64)|k(4x64)|v(4x64)]
            woutT_sb = const.tile([128, NPAIR, C], BF16)      # W_out^T rows per head pair
            qk_t = [const.tile([128, 4, TT], BF16, name=f"qk_t{i}") for i in range(NTT)]  # [qp0|qp1|kp0|kp1]
            v_t = [const.tile([128, 4, NH, DH + 64], BF16, name=f"v_t{i}") for i in range(NTT)]  # V + 64 ones cols
            o_t = [const.tile([128, NPAIR, TT], BF16, name=f"o_t{i}") for i in range(NTT)]  # normalized O^T

            for tt in range(NTT):
                nc.vector.memset(v_t[tt][:, :, :, DH:DH + 64].bitcast(U16), ONE_BF16)

            # ---- DMAs: first tile's deps first ----
            for ci in range(CK):
                nc.sync.dma_start(wqkvT_sb[:, ci, 0:FQK],
                                  wqkvT[ci * 128:(ci + 1) * 128, 0:FQK])
                nc.sync.dma_start(xT_sb[:, ci, 0:TT], xT[ci * 128:(ci + 1) * 128, 0:TT])
            for ci in range(CK):
                nc.sync.dma_start(wqkvT_sb[:, ci, FQK:FQK + FV],
                                  wqkvT[ci * 128:(ci + 1) * 128, FQK:FQK + FV])
            for tt in range(1, NTT):
                for ci in range(CK):
                    nc.sync.dma_start(xT_sb[:, ci, tt * TT:(tt + 1) * TT],
                                      xT[ci * 128:(ci + 1) * 128, tt * TT:(tt + 1) * TT])
            for pr in range(NPAIR):
                nc.sync.dma_start(woutT_sb[:, pr, :], woutT[pr * 128:(pr + 1) * 128, :])

            def phase_a(tt):
                """QKV projection for t-tile tt."""
                tsl = slice(tt * TT, (tt + 1) * TT)
                for fq in range(2):  # 0: q pairs, 1: k pairs
                    ps = psS.tile([128, 2, TT], F32, tag="ps", name=f"qk{tt}_{fq}")
                    for f2 in range(2):
                        f = 2 * fq + f2
                        for ci in range(CK):
                            nc.tensor.matmul(
                                ps[:, f2, :],
                                wqkvT_sb[:, ci, f * 128:(f + 1) * 128],
                                xT_sb[:, ci, tsl],
                                start=(ci == 0), stop=(ci == CK - 1),
                            )
                    nc.scalar.copy(qk_t[tt][:, 2 * fq:2 * fq + 2, :], ps)
                for sp in range(2):
                    pvv = [psS.tile([128, FV], F32, tag="ps", name=f"v{tt}_{sp}_{k}")
                           for k in range(2)]
                    for k in range(2):
                        si = tt * 4 + sp * 2 + k
                        for ci in range(CK):
                            nc.tensor.matmul(
                                pvv[k],
                                xT_sb[:, ci, si * 128:(si + 1) * 128],
                                wqkvT_sb[:, ci, FQK:FQK + FV],
                                start=(ci == 0), stop=(ci == CK - 1),
                            )
                    for k in range(2):
                        nc.scalar.copy(
                            v_t[tt][:, sp * 2 + k, :, 0:DH],
                            pvv[k].rearrange("p (h d) -> p h d", h=NH),
                        )

            def phase_b(tt, pr, pending):
                """Attention for (t-tile, head pair), depth-1 pipelined.
                `pending` (deferred norm emitter) is flushed after the first
                QK+exp so its broadcast matmul never stalls the PE queue."""
                n_ss = 4 * (tt + 1)
                pv = [psV.tile([128, TT], F32, tag="pv", name=f"pv{tt}_{pr}_{hi}")
                      for hi in range(2)]

                def emit_pv(pt, t0, ss):
                    for hi in range(2):
                        nc.tensor.matmul(
                            pv[hi][:, t0:TT],
                            v_t[ss // 4][:, ss % 4, pr * 2 + hi, :],
                            pt[:, hi, t0:TT],
                            start=(ss == 0), stop=(ss == n_ss - 1),
                            skip_group_check=True,
                        )

                prev = None
                for ss in range(n_ss):
                    t0 = max(0, 128 * ss - TT * tt)
                    ps = psS.tile([128, 2, TT], F32, tag="ps", name=f"s{tt}_{pr}_{ss}")
                    for hi in range(2):
                        nc.tensor.matmul(
                            ps[:, hi, t0:TT],
                            qk_t[ss // 4][hi * 64:(hi + 1) * 64, 2 + pr,
                                          (ss % 4) * 128:(ss % 4 + 1) * 128],
                            qk_t[tt][hi * 64:(hi + 1) * 64, pr, t0:TT],
                        )
                    pt = ptp.tile([128, 2, TT], BF16, tag="pt", name=f"pt{tt}_{pr}_{ss}")
                    nc.scalar.activation(pt[:, :, t0:TT], ps[:, :, t0:TT], EXP, scale=0.125)
                    if ss >= 4 * tt:  # diagonal chunk: zero the s > t triangle
                        for hi in range(2):
                            nc.gpsimd.affine_select(
                                out=pt[:, hi, t0:t0 + 128],
                                in_=pt[:, hi, t0:t0 + 128],
                                compare_op=mybir.AluOpType.is_ge,
                                fill=0.0,
                                base=0,
                                channel_multiplier=-1,
                                pattern=[[1, 128]],
                            )
                    if pending is not None and ss == 1:
                        pending()
                        pending = None
                    if prev is not None:
                        emit_pv(*prev)
                    prev = (pt, t0, ss)
                emit_pv(*prev)
                if pending is not None:
                    pending()
                return pv

            def norm(tt, pr, pv):
                """o = pv[0:64] / L; pv[64:128] all hold L (64 ones cols in
                v), so one 64-wide reciprocal IS the partition broadcast."""
                for hi in range(2):
                    lc = bcp.tile([64, TT], F32, tag="lc", name=f"lc{tt}_{pr}_{hi}")
                    nc.vector.tensor_copy(lc, pv[hi][64:128, :])
                    rc = rcp.tile([64, TT], F32, tag="rcr", name=f"rc{tt}_{pr}_{hi}")
                    nc.vector.reciprocal_approx_fast(out=rc, in_=lc)
                    nc.vector.tensor_mul(
                        o_t[tt][hi * 64:(hi + 1) * 64, pr, :],
                        pv[hi][0:DH, :],
                        rc,
                    )

            def outproj(tt):
                """y[tt-slice] = sum_pr o_t[tt]^T @ woutT."""
                for tq in range(4):
                    ps = psS.tile([128, 2, TT], F32, tag="ps", name=f"y{tt}_{tq}")
                    for ot in range(2):
                        for pr in range(NPAIR):
                            nc.tensor.matmul(
                                ps[:, ot, :],
                                o_t[tt][:, pr, tq * 128:(tq + 1) * 128],
                                woutT_sb[:, pr, ot * TT:(ot + 1) * TT],
                                start=(pr == 0), stop=(pr == NPAIR - 1),
                            )
                    yt = yp.tile([128, 2, TT], BF16, tag="yt", name=f"yt{tt}_{tq}")
                    nc.scalar.copy(yt, ps)
                    row = tt * 4 + tq
                    nc.sync.dma_start(
                        y[row * 128:(row + 1) * 128, :],
                        yt.rearrange("p a b -> p (a b)"),
                    )

            pending = None
            for tt in range(NTT):
                phase_a(tt)
                pv0 = phase_b(tt, 0, pending)  # flushes norm(tt-1, 1)
                if tt > 0:
                    outproj(tt - 1)
                pending = (lambda a=tt, b=pv0: norm(a, 0, b))
                pv1 = phase_b(tt, 1, pending)  # flushes norm(tt, 0)
                pending = (lambda a=tt, b=pv1: norm(a, 1, b))
            pending()
            outproj(NTT - 1)

    nc.compile()
    return nc


_NC_CACHE = None


def _get_nc():
    global _NC_CACHE
    if _NC_CACHE is None:
        _NC_CACHE = build_nc()
    return _NC_CACHE


def make_in_maps(x, W_qkv, W_out):
    bf = ml_dtypes.bfloat16
    x = np.asarray(x, dtype=np.float32)
    W_qkv = np.asarray(W_qkv, dtype=np.float32)
    W_out = np.asarray(W_out, dtype=np.float32)
    xT = [np.ascontiguousarray(x[b].T.astype(bf)) for b in range(B)]
    in_maps = []
    for c in range(NCORES):
        b, g = c // 4, c % 4
        rq = W_qkv[g * 256:(g + 1) * 256]            # q rows, heads 4g..4g+3
        rk = W_qkv[C + g * 256:C + (g + 1) * 256]    # k rows
        rv = W_qkv[2 * C + g * 256:2 * C + (g + 1) * 256]  # v rows
        wqkvT = np.ascontiguousarray(
            np.concatenate([rq, rk, rv], axis=0).T.astype(bf))
        woutT = np.ascontiguousarray(W_out[:, g * 256:(g + 1) * 256].T.astype(bf))
        in_maps.append({"xT": xT[b], "wqkvT": wqkvT, "woutT": woutT})
    return in_maps


def kernel(x, W_qkv, W_out):
    nc = _get_nc()
    in_maps = make_in_maps(x, W_qkv, W_out)
    res = run_bass_kernel_spmd(nc, in_maps, core_ids=list(range(NCORES)))
    kernel.last_results = res
    y = np.zeros((B, T, C), dtype=np.float32)
    for c in range(NCORES):
        y[c // 4] += res.results[c]["y"].astype(np.float32)
    return y
